# revision 49
# baseline (speedup 1.0000x reference)
"""Single-head causal attention (B=8, T=2048, C=768, H=64) on 8 TRN2 NeuronCores.

Sharding: data-parallel over the batch dim - one batch element per core.

Per-core algorithm (bf16 matmul operands, fp32 PSUM accumulation):
  - xT [C, T] bf16 fed from host; DMA'd in four 512-column stripes, each split
    across both HWDGE queues. Warmup matmuls fill the PE while the first
    stripes stream in so the HAM clock gate is released when real work lands.
    The exp activation table is preloaded via a dummy activation in the same
    dead time.
  - qkT [128, T]: rows 0:64 = q^T, 64:128 = k^T (fused [Wq | Wk] weights,
    xT chunk moving). Stripe 0's k^T is computed directly at base partition 0
    via a second K-only projection (the PE is DMA-starved at the head anyway);
    the other stripes' k^T halves are shifted to the base-0 tile via
    SBUF->SBUF DMA on the gpsimd software-DGE queue (off the FIFO hardware
    queues that carry x).
  - v computed directly in NATURAL layout (no transposes): per (t-chunk, c):
    lhsT = xT chunk [128c, 128t] (stationary, FWL), rhs = Wv [128c, 64] moving
    -> psum [128t, 64].
  - attention in S^T layout (keys j on partitions, queries i on free):
    S^T(j-chunk, i-range) = kT_j.T @ qT, 1024-wide column groups. Scale+exp
    fused on ScalarE (PSUM -> SBUF bf16). Causal: only j <= i blocks are
    computed; leading 128-col diagonal block gets an upper-tri mask multiply.
  - AV with probs STATIONARY: out_nat[i-chunk] += prb[:, i-chunk].T @ [v_j|1]
    (65 moving cols per (j,i-chunk) step - half the streaming columns of the
    v-stationary form) accumulating natural-layout [128, 65] psum regions.
    Row... col 64 accumulates the softmax denominators for free. Pair (g,jj)
    completes output chunk i=jj (its diagonal), so finalize (reciprocal of
    col 64 + tensor_scalar multiply -> bf16) streams through the whole
    attention phase and the tail after the last exp is tiny. One merged
    output DMA per 4 chunks.
  - QKV projection stripes 2,3 interleave into the group-0 attention pipeline
    so the PE, ScalarE (exp), and DVE all stay busy concurrently.

No max-subtraction in softmax: scores * C**-0.5 are bounded (|s| < ~3), exp is
safe in fp32, and the result is mathematically identical to jax.nn.softmax.
Output is bf16 on-device (rel err ~6e-3 total), upcast to f32 on the host.
"""

import ml_dtypes
import numpy as np

import concourse.bass as bass
import concourse.tile as tile
from concourse import bacc, mybir
from concourse.bass import ds, ts
from concourse.masks import make_upper_triangular

B, T, C, H = 8, 2048, 768, 64
P = 128
NCH = C // P          # 6 contraction chunks for QKV
GW = 1024             # attention output column-group width
NG = T // GW          # 2 groups
NT = T // P           # 16 t-chunks
JPG = GW // P         # 8 j-chunks per group
SCALE = float(C) ** -0.5

F32 = mybir.dt.float32
BF16 = mybir.dt.bfloat16
EXP = mybir.ActivationFunctionType.Exp
FP8 = mybir.dt.float8e4


def _emit(tc: tile.TileContext, ctx, xT, x8, wqk, wv, out):
    nc = tc.nc

    consts = ctx.enter_context(tc.tile_pool(name="consts", bufs=1))
    xpool = ctx.enter_context(tc.tile_pool(name="x", bufs=1))
    qpool = ctx.enter_context(tc.tile_pool(name="qkv", bufs=1))

    tri = consts.tile([P, P], BF16)
    make_upper_triangular(nc, tri[:], val=1.0, diag=True)
    scratch = consts.tile([1, 1], F32)

    # weights split across the two HWDGE queues to balance the head load
    w_qk = consts.tile([P, NCH, P], FP8)
    nc.scalar.dma_start(w_qk[:], wqk.rearrange("(o p) m -> p o m", p=P))
    w_v = consts.tile([P, NCH, H], BF16)
    nc.sync.dma_start(w_v[:], wv.rearrange("(o p) m -> p o m", p=P))

    # xT in four 512-col stripes, each split across the two HWDGE queues.
    # The host pre-tiles x as [p, stripe, half, 3*512] so each stripe piece
    # is one fully-contiguous 3KB run per partition (fast DMA descriptors).
    xT_sb = xpool.tile([P, 4, NCH, 512], BF16)
    x8_sb = xpool.tile([P, 4, NCH, 512], FP8)
    # queue order: fp8 stripe 0, bf16 stripe 0 (for the first v chunks), then
    # the remaining fp8 stripes (unblocks every QK/S chunk early), then the
    # remaining bf16 stripes for the trailing v projections
    for s in range(4):
        nc.scalar.dma_start(x8_sb[:, s, 0:3, :], x8[:, s, 0, :])
        nc.sync.dma_start(x8_sb[:, s, 3:6, :], x8[:, s, 1, :])
        if s == 1:
            nc.scalar.dma_start(xT_sb[:, 0, 0:3, :], xT[:, 0, 0, :])
            nc.sync.dma_start(xT_sb[:, 0, 3:6, :], xT[:, 0, 1, :])
        if s == 1:
            # preload the exp activation table during the DMA dead time so the
            # first real exp doesn't pay the ~2.7us table-load cost
            nc.scalar.activation(scratch[:], scratch[:], EXP)
    for s in range(1, 4):
        nc.scalar.dma_start(xT_sb[:, s, 0:3, :], xT[:, s, 0, :])
        nc.sync.dma_start(xT_sb[:, s, 3:6, :], xT[:, s, 1, :])

    qkT = qpool.tile([P, T], BF16)
    kT = qpool.tile([H, T], BF16)
    v_sb = qpool.tile([P, NT, H + 1], BF16)
    nc.vector.memset(v_sb[:, :, H : H + 1], 1.0)

    # warmup: dummy matmuls fill the PE while the first x stripes stream in,
    # so the HAM clock gate is already released when real work arrives
    dum = qpool.tile([P, 512], BF16)
    nc.vector.memset(dum[:], 0.0)
    with tc.tile_pool(name="warm", bufs=2, space="PSUM") as wp:
        for w in range(14):
            dps = wp.tile([P, 512], F32, tag="w", name=f"warm_{w}")
            nc.tensor.matmul(dps[:], dum[:, 0:P], dum[:], start=True, stop=True)

    # shared PSUM pools: sp serves both projection stripes and S^T chunks
    sp = ctx.enter_context(tc.tile_pool(name="spsum", bufs=3, space="PSUM"))
    op = ctx.enter_context(tc.tile_pool(name="opsum", bufs=2, space="PSUM"))
    pb = ctx.enter_context(tc.tile_pool(name="probs", bufs=8))
    fin = ctx.enter_context(tc.tile_pool(name="fin", bufs=3))

    def emit_proj_qk(s):
        # one 512-col stripe of q|k transposed (fused [Wq | Wk] weights)
        ps = sp.tile([P, GW], F32, tag="s", name=f"projqk_{s}")
        for c in range(NCH):
            nc.tensor.matmul(
                ps[:, 0:512],
                w_qk[:, c, :],
                x8_sb[:, s, c, :],
                start=(c == 0),
                stop=(c == NCH - 1),
            )
        nc.vector.tensor_copy(qkT[:, ts(s, 512)], ps[:, 0:512])
        if s == 0:
            # stripe 0 gates the whole attention pipeline: compute its k^T
            # directly at base partition 0 via a second K-only projection
            # (PE is DMA-starved at the head anyway) instead of waiting for
            # the qkT copy + SBUF->SBUF shift round trip
            for c in range(NCH):
                nc.tensor.matmul(
                    ps[0:H, 512:1024],
                    w_qk[:, c, ds(H, H)],
                    x8_sb[:, 0, c, :],
                    start=(c == 0),
                    stop=(c == NCH - 1),
                )
            nc.vector.tensor_copy(kT[:, 0:512], ps[0:H, 512:1024])
        else:
            # k^T shift to base partition 0 on the gpsimd software-DGE queue:
            # keeps it off the FIFO hardware queues that carry the bulk x
            nc.gpsimd.dma_start(kT[:, ts(s, 512)], qkT[H:P, ts(s, 512)])

    def emit_proj_v(s, half=None):
        # v in natural layout: xT chunk stationary (FWL), Wv moving.
        # half=0/1 emits only 2 t-chunks: shorter PE-queue blocks when
        # interleaved between attention S^T chunks.
        rng = range(4) if half is None else range(2 * half, 2 * half + 2)
        ps = sp.tile([P, GW], F32, tag="s", name=f"projv_{s}_{half}")
        for i in rng:
            t = 4 * s + i
            reg = ps[:, ds(H * i, H)]
            for c in range(NCH):
                nc.tensor.matmul(
                    reg,
                    xT_sb[:, s, c, ds(i * P, P)],
                    w_v[:, c, :],
                    start=(c == 0),
                    stop=(c == NCH - 1),
                )
            nc.vector.tensor_copy(v_sb[:, t, 0:H], reg)

    def emit_probs(g, jj, c0=0, c1=None):
        istart = max(g * GW, jj * P) + c0
        n = ((g + 1) * GW if c1 is None else max(g * GW, jj * P) + c1) - istart
        sps = sp.tile([P, GW], F32, tag="s")
        for h in range(0, n, 512):
            nh = min(512, n - h)
            nc.tensor.matmul(
                sps[:, h : h + nh],
                kT[:, ts(jj, P)],
                qkT[0:H, ds(istart + h, nh)],
                start=True,
                stop=True,
            )
        prb = pb.tile([P, GW], BF16, tag="p")
        nc.scalar.activation(prb[:, :n], sps[:, :n], EXP, scale=SCALE)
        if jj >= JPG * g and c0 == 0:
            # leading 128 cols are the diagonal block: upper-tri (j<=i) mask
            nc.vector.tensor_mul(out=prb[:, :P], in0=prb[:, :P], in1=tri[:])
        return prb

    # QK stripe 0 first, then the first half of S^T(0,0) (it needs only
    # stripe 0's q and the direct k^T), so the ScalarE exp stream starts one
    # QK-stripe earlier; then QK stripe 1 and the rest.
    emit_proj_qk(0)

    # remaining projection units interleave into group-0 attention; they must
    # be emitted on the PE queue before anything that consumes their outputs,
    # but late enough that their input data has landed by the time the PE
    # reaches them (avoids head-of-line blocking + a HAM warmup reset)
    inject = {
        1: [("v", 0, 0)],
        2: [("v", 0, 1)],
        3: [("qk", 2)],
        4: [("qk", 3)],
        5: [("v", 1, 0)],
        6: [("v", 1, 1)],
        10: [("v", 2, 0)],
        12: [("v", 2, 1)],
        14: [("v", 3, 0)],
        16: [("v", 3, 1)],
    }

    ops_by_g = {}
    onat_by_q = {}
    LOOKAHEAD = 2
    # flat pair list with lookahead ACROSS the group boundary: group 1's
    # first S^T chunks (and their exps) are emitted while group-0 AV work is
    # still in the PE queue, so the scalar exp stream never pauses at the
    # boundary. The inject table guarantees QK(3) precedes pairs[8]=(1,0).
    pairs = [(g, jj) for g in range(NG) for jj in range(JPG * g + JPG)]
    prb00a = emit_probs(0, 0, 0, 512)
    emit_proj_qk(1)
    prb00b = emit_probs(0, 0, 512, GW)
    prb_queue = [(prb00a, prb00b), emit_probs(0, 1)]
    DELAY = 3

    def emit_av_fin(prb, g, jj):
        if True:
            if jj == 0:
                # two half-group tiles: a [128, 8, 65] f32 region would
                # straddle a 2KB PSUM bank boundary, which a matmul
                # accumulation region must not cross
                ops_by_g[g] = [
                    op.tile([P, 4, H + 1], F32, tag="o", name=f"ops_{g}_{hh}")
                    for hh in range(2)
                ]
            istart = max(g * GW, jj * P)
            # AV with probs stationary: one 65-col matmul per output i-chunk,
            # accumulating natural-layout [128, 65] psum regions
            # start=True clears the has_written bits of the WHOLE bank, so only
            # the first matmul into each bank may set it; the other regions
            # self-initialize via flags=0 overwrite-where-bit-unset semantics
            for ii in range(max(jj, JPG * g), JPG * g + JPG):
                il = ii - JPG * g
                if isinstance(prb, tuple):
                    pt = prb[il // 4]
                    psl = pt[:, ds((il % 4) * P, P)]
                else:
                    psl = prb[:, ds(ii * P - istart, P)]
                nc.tensor.matmul(
                    ops_by_g[g][il // 4][:, il % 4, :],
                    psl,
                    v_sb[:, jj, :],
                    start=(jj == 0 and il % 4 == 0),
                    stop=(jj == ii),
                    skip_group_check=True,
                )

            # pair (g, jj) completes output chunk i=jj (its diagonal block):
            # normalize it now so finalize streams through the whole phase
            if jj >= JPG * g:
                quad, slot = jj // 4, jj % 4
                if slot == 0:
                    onat_by_q[quad] = fin.tile(
                        [P, 4, H], BF16, tag="onat", name=f"onat_{quad}"
                    )
                o_nat = onat_by_q[quad]
                il = jj - JPG * g
                reg = ops_by_g[g][il // 4][:, il % 4, :]
                rch = fin.tile([P, 1], F32, tag="rch")
                nc.vector.reciprocal(rch[:], reg[:, H : H + 1])
                nc.vector.tensor_scalar_mul(o_nat[:, slot, :], reg[:, 0:H], rch[:])
                if slot == 3:
                    ov = out.rearrange("(a p) h -> p a h", p=P)
                    nc.sync.dma_start(ov[:, ds(quad * 4, 4), :], o_nat[:])

    # the S/exp stream runs DELAY pairs ahead of the AV/finalize stream, so
    # S chunk emission (which paces ScalarE) never head-of-line blocks on the
    # later-arriving bf16 v data, and each AV's exp is long finished when the
    # PE reaches it
    pending = []
    for idx, (g, jj) in enumerate(pairs):
        prb = prb_queue.pop(0)
        if idx + LOOKAHEAD < len(pairs):
            prb_queue.append(emit_probs(*pairs[idx + LOOKAHEAD]))
        pending.append((prb, g, jj))
        for kind, *args in inject.get(idx, ()):
            if kind == "qk":
                emit_proj_qk(*args)
            else:
                emit_proj_v(*args)
        if len(pending) > DELAY:
            emit_av_fin(*pending.pop(0))
    for p in pending:
        emit_av_fin(*p)


def build():
    from contextlib import ExitStack

    nc = bacc.Bacc("TRN2", target_bir_lowering=False, debug=False, num_devices=B)
    xT = nc.dram_tensor("xT", [P, 4, 2, 3 * 512], BF16, kind="ExternalInput").ap()
    x8 = nc.dram_tensor("x8", [P, 4, 2, 3 * 512], FP8, kind="ExternalInput").ap()
    wqk = nc.dram_tensor("wqk", [C, P], FP8, kind="ExternalInput").ap()
    wv = nc.dram_tensor("wv", [C, H], BF16, kind="ExternalInput").ap()
    out = nc.dram_tensor("o", [T, H], BF16, kind="ExternalOutput").ap()
    with tile.TileContext(nc) as tc, ExitStack() as ctx:
        _emit(tc, ctx, xT, x8, wqk, wv, out)
    nc.compile()
    return nc


_NC = None


def _get_nc():
    global _NC
    if _NC is None:
        _NC = build()
    return _NC


def make_in_maps(x, Wk, Wq, Wv):
    bf = ml_dtypes.bfloat16
    wqk = np.ascontiguousarray(np.concatenate([Wq, Wk], axis=1)).astype(
        ml_dtypes.float8_e4m3fn
    )
    wv = np.ascontiguousarray(np.asarray(Wv)).astype(bf)
    def tile_x(xb, dt):
        # [C, T] -> [p, stripe, half, 3*512]: per-partition-contiguous stripe
        # pieces so each DMA descriptor covers a 3KB run
        xt = np.asarray(xb).T.reshape(NCH, P, 4, 512).transpose(1, 2, 0, 3)
        return np.ascontiguousarray(xt.reshape(P, 4, 2, 3 * 512)).astype(dt)

    f8 = ml_dtypes.float8_e4m3fn
    return [
        {
            "xT": tile_x(x[b], bf),
            "x8": tile_x(x[b], f8),
            "wqk": wqk,
            "wv": wv,
        }
        for b in range(B)
    ]


def kernel(x, Wk, Wq, Wv):
    from concourse.bass_utils import run_bass_kernel_spmd

    nc = _get_nc()
    in_maps = make_in_maps(x, Wk, Wq, Wv)
    r = run_bass_kernel_spmd(nc, in_maps, core_ids=list(range(B)))
    out = np.stack([np.asarray(r.results[b]["o"]) for b in range(B)])
    return np.ascontiguousarray(out.astype(np.float32))


# revision 51
# speedup vs baseline: 1.0709x; 1.0709x over previous
"""Single-head causal attention (B=8, T=2048, C=768, H=64) on 8 TRN2 NeuronCores.

Sharding: data-parallel over the batch dim - one batch element per core.

Per-core algorithm (bf16 matmul operands, fp32 PSUM accumulation):
  - xT [C, T] bf16 fed from host; DMA'd in four 512-column stripes, each split
    across both HWDGE queues. Warmup matmuls fill the PE while the first
    stripes stream in so the HAM clock gate is released when real work lands.
    The exp activation table is preloaded via a dummy activation in the same
    dead time.
  - qkT [128, T]: rows 0:64 = q^T, 64:128 = k^T (fused [Wq | Wk] weights,
    xT chunk moving). Stripe 0's k^T is computed directly at base partition 0
    via a second K-only projection (the PE is DMA-starved at the head anyway);
    the other stripes' k^T halves are shifted to the base-0 tile via
    SBUF->SBUF DMA on the gpsimd software-DGE queue (off the FIFO hardware
    queues that carry x).
  - v computed directly in NATURAL layout (no transposes): per (t-chunk, c):
    lhsT = xT chunk [128c, 128t] (stationary, FWL), rhs = Wv [128c, 64] moving
    -> psum [128t, 64].
  - attention in S^T layout (keys j on partitions, queries i on free):
    S^T(j-chunk, i-range) = kT_j.T @ qT, 1024-wide column groups. Scale+exp
    fused on ScalarE (PSUM -> SBUF bf16). Causal: only j <= i blocks are
    computed; leading 128-col diagonal block gets an upper-tri mask multiply.
  - AV with probs STATIONARY: out_nat[i-chunk] += prb[:, i-chunk].T @ [v_j|1]
    (65 moving cols per (j,i-chunk) step - half the streaming columns of the
    v-stationary form) accumulating natural-layout [128, 65] psum regions.
    Row... col 64 accumulates the softmax denominators for free. Pair (g,jj)
    completes output chunk i=jj (its diagonal), so finalize (reciprocal of
    col 64 + tensor_scalar multiply -> bf16) streams through the whole
    attention phase and the tail after the last exp is tiny. One merged
    output DMA per 4 chunks.
  - QKV projection stripes 2,3 interleave into the group-0 attention pipeline
    so the PE, ScalarE (exp), and DVE all stay busy concurrently.

No max-subtraction in softmax: scores * C**-0.5 are bounded (|s| < ~3), exp is
safe in fp32, and the result is mathematically identical to jax.nn.softmax.
Output is bf16 on-device (rel err ~6e-3 total), upcast to f32 on the host.
"""

import ml_dtypes
import numpy as np

import concourse.bass as bass
import concourse.tile as tile
from concourse import bacc, mybir
from concourse.bass import ds, ts
from concourse.masks import make_upper_triangular

B, T, C, H = 8, 2048, 768, 64
P = 128
NCH = C // P          # 6 contraction chunks for QKV
GW = 1024             # attention output column-group width
NG = T // GW          # 2 groups
NT = T // P           # 16 t-chunks
JPG = GW // P         # 8 j-chunks per group
SCALE = float(C) ** -0.5

F32 = mybir.dt.float32
BF16 = mybir.dt.bfloat16
EXP = mybir.ActivationFunctionType.Exp
FP8 = mybir.dt.float8e4


def _emit(tc: tile.TileContext, ctx, xT, x8, wqk, wv, out):
    nc = tc.nc

    consts = ctx.enter_context(tc.tile_pool(name="consts", bufs=1))
    xpool = ctx.enter_context(tc.tile_pool(name="x", bufs=1))
    qpool = ctx.enter_context(tc.tile_pool(name="qkv", bufs=1))

    tri = consts.tile([P, P], BF16)
    make_upper_triangular(nc, tri[:], val=1.0, diag=True)
    scratch = consts.tile([1, 1], F32)

    # weights split across the two HWDGE queues to balance the head load
    w_qk = consts.tile([P, NCH, P], FP8)
    nc.scalar.dma_start(w_qk[:], wqk.rearrange("(o p) m -> p o m", p=P))
    w_v = consts.tile([P, NCH, H], BF16)
    nc.sync.dma_start(w_v[:], wv.rearrange("(o p) m -> p o m", p=P))

    # xT in four 512-col stripes, each split across the two HWDGE queues.
    # The host pre-tiles x as [p, stripe, half, 3*512] so each stripe piece
    # is one fully-contiguous 3KB run per partition (fast DMA descriptors).
    xT_sb = xpool.tile([P, 4, NCH, 512], BF16)
    x8_sb = xpool.tile([P, 4, NCH, 512], FP8)
    # queue order: fp8 stripe 0, bf16 stripe 0 (for the first v chunks), then
    # the remaining fp8 stripes (unblocks every QK/S chunk early), then the
    # remaining bf16 stripes for the trailing v projections
    for s in range(4):
        nc.scalar.dma_start(x8_sb[:, s, 0:3, :], x8[:, s, 0, :])
        nc.sync.dma_start(x8_sb[:, s, 3:6, :], x8[:, s, 1, :])
        if s == 1:
            nc.scalar.dma_start(xT_sb[:, 0, 0:3, :], xT[:, 0, 0, :])
            nc.sync.dma_start(xT_sb[:, 0, 3:6, :], xT[:, 0, 1, :])
        if s == 1:
            # preload the exp activation table during the DMA dead time so the
            # first real exp doesn't pay the ~2.7us table-load cost
            nc.scalar.activation(scratch[:], scratch[:], EXP)
    for s in range(1, 4):
        nc.scalar.dma_start(xT_sb[:, s, 0:3, :], xT[:, s, 0, :])
        nc.sync.dma_start(xT_sb[:, s, 3:6, :], xT[:, s, 1, :])

    qkT = qpool.tile([P, T], BF16)
    kT = qpool.tile([H, T], BF16)
    v_sb = qpool.tile([P, NT, H + 1], BF16)
    nc.vector.memset(v_sb[:, :, H : H + 1], 1.0)

    # warmup: dummy matmuls fill the PE while the first x stripes stream in,
    # so the HAM clock gate is already released when real work arrives
    dum = qpool.tile([P, 512], BF16)
    nc.vector.memset(dum[:], 0.0)
    with tc.tile_pool(name="warm", bufs=2, space="PSUM") as wp:
        for w in range(14):
            dps = wp.tile([P, 512], F32, tag="w", name=f"warm_{w}")
            nc.tensor.matmul(dps[:], dum[:, 0:P], dum[:], start=True, stop=True)

    # shared PSUM pools: sp serves both projection stripes and S^T chunks
    sp = ctx.enter_context(tc.tile_pool(name="spsum", bufs=3, space="PSUM"))
    op = ctx.enter_context(tc.tile_pool(name="opsum", bufs=2, space="PSUM"))
    pb = ctx.enter_context(tc.tile_pool(name="probs", bufs=6))
    fin = ctx.enter_context(tc.tile_pool(name="fin", bufs=3))

    def emit_proj_qk(s):
        # one 512-col stripe of q|k transposed (fused [Wq | Wk] weights)
        ps = sp.tile([P, GW], F32, tag="s", name=f"projqk_{s}")
        for c in range(NCH):
            nc.tensor.matmul(
                ps[:, 0:512],
                w_qk[:, c, :],
                x8_sb[:, s, c, :],
                start=(c == 0),
                stop=(c == NCH - 1),
            )
        nc.vector.tensor_copy(qkT[:, ts(s, 512)], ps[:, 0:512])
        if s == 0:
            # stripe 0 gates the whole attention pipeline: compute its k^T
            # directly at base partition 0 via a second K-only projection
            # (PE is DMA-starved at the head anyway) instead of waiting for
            # the qkT copy + SBUF->SBUF shift round trip
            for c in range(NCH):
                nc.tensor.matmul(
                    ps[0:H, 512:1024],
                    w_qk[:, c, ds(H, H)],
                    x8_sb[:, 0, c, :],
                    start=(c == 0),
                    stop=(c == NCH - 1),
                )
            # ScalarE is idle before the first exp: do this copy there, in
            # parallel with the vector qkT copy, to shorten the critical
            # chain to the first S^T chunk
            nc.scalar.copy(kT[:, 0:512], ps[0:H, 512:1024])
        else:
            # k^T shift to base partition 0 on the gpsimd software-DGE queue:
            # keeps it off the FIFO hardware queues that carry the bulk x
            nc.gpsimd.dma_start(kT[:, ts(s, 512)], qkT[H:P, ts(s, 512)])

    def emit_proj_v(s, half=None):
        # v in natural layout: xT chunk stationary (FWL), Wv moving.
        # half=0/1 emits only 2 t-chunks: shorter PE-queue blocks when
        # interleaved between attention S^T chunks.
        rng = range(4) if half is None else range(2 * half, 2 * half + 2)
        ps = sp.tile([P, GW], F32, tag="s", name=f"projv_{s}_{half}")
        for i in rng:
            t = 4 * s + i
            reg = ps[:, ds(H * i, H)]
            for c in range(NCH):
                nc.tensor.matmul(
                    reg,
                    xT_sb[:, s, c, ds(i * P, P)],
                    w_v[:, c, :],
                    start=(c == 0),
                    stop=(c == NCH - 1),
                )
            nc.vector.tensor_copy(v_sb[:, t, 0:H], reg)

    def emit_probs(g, jj, c0=0, c1=None):
        istart = max(g * GW, jj * P) + c0
        n = ((g + 1) * GW if c1 is None else max(g * GW, jj * P) + c1) - istart
        sps = sp.tile([P, GW], F32, tag="s")
        for h in range(0, n, 512):
            nh = min(512, n - h)
            nc.tensor.matmul(
                sps[:, h : h + nh],
                kT[:, ts(jj, P)],
                qkT[0:H, ds(istart + h, nh)],
                start=True,
                stop=True,
            )
        prb = pb.tile([P, GW], BF16, tag="p")
        nc.scalar.activation(prb[:, :n], sps[:, :n], EXP, scale=SCALE)
        if jj >= JPG * g and c0 == 0:
            # leading 128 cols are the diagonal block: upper-tri (j<=i) mask
            nc.vector.tensor_mul(out=prb[:, :P], in0=prb[:, :P], in1=tri[:])
        return prb

    # QK stripe 0 first, then the first half of S^T(0,0) (it needs only
    # stripe 0's q and the direct k^T), so the ScalarE exp stream starts one
    # QK-stripe earlier; then QK stripe 1 and the rest.
    emit_proj_qk(0)

    # remaining projection units interleave into group-0 attention; they must
    # be emitted on the PE queue before anything that consumes their outputs,
    # but late enough that their input data has landed by the time the PE
    # reaches them (avoids head-of-line blocking + a HAM warmup reset)
    inject = {
        0: [("v", 1, 0)],
        1: [("v", 1, 1)],
        3: [("qk", 2)],
        4: [("v", 2, 0)],
        5: [("qk", 3)],
        6: [("v", 2, 1)],
        7: [("v", 3, 0)],
        8: [("v", 3, 1)],
    }

    ops_by_g = {}
    onat_by_q = {}
    LOOKAHEAD = 2
    # flat pair list with lookahead ACROSS the group boundary: group 1's
    # first S^T chunks (and their exps) are emitted while group-0 AV work is
    # still in the PE queue, so the scalar exp stream never pauses at the
    # boundary. The inject table guarantees QK(3) precedes pairs[8]=(1,0).
    pairs = [(g, jj) for g in range(NG) for jj in range(JPG * g + JPG)]
    prb00a = emit_probs(0, 0, 0, 512)
    emit_proj_qk(1)
    prb00b = emit_probs(0, 0, 512, GW)
    prb_queue = [(prb00a, prb00b), emit_probs(0, 1)]
    emit_proj_v(0)
    for idx, (g, jj) in enumerate(pairs):
        if True:
            prb = prb_queue.pop(0)
            if idx + LOOKAHEAD < len(pairs):
                prb_queue.append(emit_probs(*pairs[idx + LOOKAHEAD]))

            if jj == 0:
                # two half-group tiles: a [128, 8, 65] f32 region would
                # straddle a 2KB PSUM bank boundary, which a matmul
                # accumulation region must not cross
                ops_by_g[g] = [
                    op.tile([P, 4, H + 1], F32, tag="o", name=f"ops_{g}_{hh}")
                    for hh in range(2)
                ]
            istart = max(g * GW, jj * P)
            # AV with probs stationary: one 65-col matmul per output i-chunk,
            # accumulating natural-layout [128, 65] psum regions
            # start=True clears the has_written bits of the WHOLE bank, so only
            # the first matmul into each bank may set it; the other regions
            # self-initialize via flags=0 overwrite-where-bit-unset semantics
            for ii in range(max(jj, JPG * g), JPG * g + JPG):
                il = ii - JPG * g
                if isinstance(prb, tuple):
                    pt = prb[il // 4]
                    psl = pt[:, ds((il % 4) * P, P)]
                else:
                    psl = prb[:, ds(ii * P - istart, P)]
                nc.tensor.matmul(
                    ops_by_g[g][il // 4][:, il % 4, :],
                    psl,
                    v_sb[:, jj, :],
                    start=(jj == 0 and il % 4 == 0),
                    stop=(jj == ii),
                    skip_group_check=True,
                )

            for kind, *args in inject.get(idx, ()):
                if kind == "qk":
                    emit_proj_qk(*args)
                else:
                    emit_proj_v(*args)

            # pair (g, jj) completes output chunk i=jj (its diagonal block):
            # normalize it now so finalize streams through the whole phase
            if jj >= JPG * g:
                quad, slot = jj // 4, jj % 4
                if slot == 0:
                    onat_by_q[quad] = fin.tile(
                        [P, 4, H], BF16, tag="onat", name=f"onat_{quad}"
                    )
                o_nat = onat_by_q[quad]
                il = jj - JPG * g
                reg = ops_by_g[g][il // 4][:, il % 4, :]
                rch = fin.tile([P, 1], F32, tag="rch")
                nc.vector.reciprocal(rch[:], reg[:, H : H + 1])
                nc.vector.tensor_scalar_mul(o_nat[:, slot, :], reg[:, 0:H], rch[:])
                if slot == 3:
                    ov = out.rearrange("(a p) h -> p a h", p=P)
                    nc.sync.dma_start(ov[:, ds(quad * 4, 4), :], o_nat[:])


def build():
    from contextlib import ExitStack

    nc = bacc.Bacc("TRN2", target_bir_lowering=False, debug=False, num_devices=B)
    xT = nc.dram_tensor("xT", [P, 4, 2, 3 * 512], BF16, kind="ExternalInput").ap()
    x8 = nc.dram_tensor("x8", [P, 4, 2, 3 * 512], FP8, kind="ExternalInput").ap()
    wqk = nc.dram_tensor("wqk", [C, P], FP8, kind="ExternalInput").ap()
    wv = nc.dram_tensor("wv", [C, H], BF16, kind="ExternalInput").ap()
    out = nc.dram_tensor("o", [T, H], BF16, kind="ExternalOutput").ap()
    with tile.TileContext(nc) as tc, ExitStack() as ctx:
        _emit(tc, ctx, xT, x8, wqk, wv, out)
    nc.compile()
    return nc


_NC = None


def _get_nc():
    global _NC
    if _NC is None:
        _NC = build()
    return _NC


def make_in_maps(x, Wk, Wq, Wv):
    bf = ml_dtypes.bfloat16
    wqk = np.ascontiguousarray(np.concatenate([Wq, Wk], axis=1)).astype(
        ml_dtypes.float8_e4m3fn
    )
    wv = np.ascontiguousarray(np.asarray(Wv)).astype(bf)
    def tile_x(xb, dt):
        # [C, T] -> [p, stripe, half, 3*512]: per-partition-contiguous stripe
        # pieces so each DMA descriptor covers a 3KB run
        xt = np.asarray(xb).T.reshape(NCH, P, 4, 512).transpose(1, 2, 0, 3)
        return np.ascontiguousarray(xt.reshape(P, 4, 2, 3 * 512)).astype(dt)

    f8 = ml_dtypes.float8_e4m3fn
    return [
        {
            "xT": tile_x(x[b], bf),
            "x8": tile_x(x[b], f8),
            "wqk": wqk,
            "wv": wv,
        }
        for b in range(B)
    ]


def kernel(x, Wk, Wq, Wv):
    from concourse.bass_utils import run_bass_kernel_spmd

    nc = _get_nc()
    in_maps = make_in_maps(x, Wk, Wq, Wv)
    r = run_bass_kernel_spmd(nc, in_maps, core_ids=list(range(B)))
    out = np.stack([np.asarray(r.results[b]["o"]) for b in range(B)])
    return np.ascontiguousarray(out.astype(np.float32))
